# revision 1
# baseline (speedup 1.0000x reference)
"""CrossNet (DCN) forward on 8 Trainium2 NeuronCores.

Reference computation (L=6 cross layers):
    x0 = inputs                                  # [B, D]
    x_{i+1} = x0 * (x_i @ w_i) + b_i + x_i       # w_i: [D,1], b_i: [D]

Algebraic factorization: by induction every layer state has the form
    x_i = x0 * c_i + beta_i
with a per-row scalar c_i ([B]) and a row-constant vector beta_i ([D]):
    beta_{i+1} = beta_i + b_i                    (data independent)
    c_{i+1} = c_i * (1 + u_i) + v_i,   u_i = x0 @ w_i,  v_i = beta_i @ w_i
    out = x0 * c_L + beta_L

So the whole network is one [B,D]@[D,6] matvec batch (u), a tiny per-row
recurrence, and one final scale-add over [B,D] — HBM traffic is read x0 +
write out, the memory-bound optimum.

Device mapping (per core, 2048 rows, tiles of 128 rows, groups of 2 tiles):

* one VectorE InstStreamTranspose (32x32 blocks, SBUF->SBUF) per
  quarter-tile: xb[(a,i), t, 32C+j] = x0[32a+j, 32C+i] in fp32.
* fp16 hi/lo split (exact to ~2^-23) on the transposed data: xh = fp16(xb)
  on ScalarE; xl' = fp16(2048*(xb-xh)) via a VectorE subtract into an fp32
  scratch + ScalarE scale-cast (prescale keeps xl' out of fp16-subnormal
  range; all fp16 writes contiguous — strided fp16 writes fault the HW).
* TensorE computes u with one fp16 matmul per packed column pair:
     lhsT[(a,i), (hl,e,a',l)] = (a==a') * W_hl[32(2*C2+e)+i, l]  ([128,96])
     rhs  = fp16 view of xb pair-slice, dims (h, j64, t)          (N=256)
  accumulated into PSUM u_ps[(hl,e,a,l), (h,j64,t)] over all C2. Slots
  where the rhs half/parity doesn't match the weight half/parity hold
  garbage that the extraction never reads; W_l/xl' products carry 2048x
  factors that the extraction scales back by 2^-11.
* u_ps quarters are PE-transposed to [(2j+t), (hl,e,a,l)]; VectorE sums
  the valid slices, runs the c recurrence, and a 0/1 selector matmul + 4
  partition-aligned diagonal copies deliver c as a [128,1] per-partition
  scalar per tile; ScalarE/GpSimd apply out = x0 * c in place; DMA stores.

Sharding: data parallel over the batch dim (spec hint), params replicated.
"""

import numpy as np

B, D, L = 16384, 4096, 6
N_CORES = 8
B_SHARD = B // N_CORES   # 2048
P = 128                  # SBUF partitions
N_TILES = B_SHARD // P   # 16 row-tiles per core
TPG = 2                  # tiles per group
N_GRP = N_TILES // TPG   # 8 groups
N_C2 = D // 64           # 64 packed column pairs
QUARTER = D // 4         # stream-transpose granularity (1024)
C2PQ = N_C2 // 4         # packed pairs per quarter (16)
ML = 4 * L               # 24 = (a, l) columns per (hl, e) slot
MW = 4 * ML              # 96 = (hl, e, a, l) stationary columns
XL_SCALE = 2048.0        # xl/Wl prescale; 2^-11 folded back in extraction

_prog_cache = {}


def _build_program(use_v: bool, use_b: bool):
    """Build the SPMD bass program for one core's shard."""
    from contextlib import ExitStack

    import concourse.bass as bass
    import concourse.tile as tile
    from concourse import bacc, mybir

    f32 = mybir.dt.float32
    f16 = mybir.dt.float16
    nc = bacc.Bacc("TRN2", target_bir_lowering=False, debug=False)

    x = nc.dram_tensor("x", [B_SHARD, D], f32, kind="ExternalInput").ap()
    # wb[(a,i), C2, (hl,e,a',l)] = (a==a') * W_hl[32*(2*C2+e) + i, l]
    wb = nc.dram_tensor("wb", [P, N_C2, MW], f16, kind="ExternalInput").ap()
    # sel[(j',t'), t, (a',j)] = (t'==t)*(j==j')  (only first 64 partitions used)
    sel = nc.dram_tensor("sel", [P, TPG, P], f32, kind="ExternalInput").ap()
    ident = nc.dram_tensor("ident", [P, P], f32, kind="ExternalInput").ap()
    if use_v:
        vb = nc.dram_tensor("vb", [P, TPG * ML], f32, kind="ExternalInput").ap()
    if use_b:
        bb = nc.dram_tensor("bb", [P, D], f32, kind="ExternalInput").ap()
    out = nc.dram_tensor("out", [B_SHARD, D], f32, kind="ExternalOutput").ap()

    with tile.TileContext(nc) as tc, ExitStack() as ctx:
        singles = ctx.enter_context(tc.tile_pool(name="singles", bufs=1))
        wb_sb = singles.tile([P, N_C2, MW], f16)
        nc.sync.dma_start(wb_sb[:], wb[:])
        sel_sb = singles.tile([P, TPG, P], f32)
        nc.sync.dma_start(sel_sb[:], sel[:])
        id_sb = singles.tile([P, P], f32)
        nc.sync.dma_start(id_sb[:], ident[:])
        if use_v:
            vb_sb = singles.tile([P, TPG * ML], f32)
            nc.sync.dma_start(vb_sb[:], vb[:])
        if use_b:
            bb_sb = singles.tile([P, D], f32)
            nc.sync.dma_start(bb_sb[:], bb[:])

        xpool = ctx.enter_context(tc.tile_pool(name="xtile", bufs=3 * TPG))
        packp = ctx.enter_context(tc.tile_pool(name="xpack", bufs=3))
        xbq = ctx.enter_context(tc.tile_pool(name="xbq", bufs=4))
        junkp = ctx.enter_context(tc.tile_pool(name="junk", bufs=2))
        upsum = ctx.enter_context(
            tc.tile_pool(name="upsum", bufs=2, space=bass.MemorySpace.PSUM)
        )
        utps = ctx.enter_context(
            tc.tile_pool(name="utps", bufs=4, space=bass.MemorySpace.PSUM)
        )
        cps = ctx.enter_context(
            tc.tile_pool(name="cps", bufs=2, space=bass.MemorySpace.PSUM)
        )
        small = ctx.enter_context(tc.tile_pool(name="small", bufs=4))

        for g in range(N_GRP):
            xts = []
            for t in range(TPG):
                xt = xpool.tile([P, D], f32, tag="xtile")
                nc.sync.dma_start(xt[:], x[(g * TPG + t) * P : (g * TPG + t + 1) * P, :])
                xts.append(xt)

            # u_ps[(hl,e,a,l), (h, j64, t)] — see module docstring.
            u_ps = upsum.tile([P, 2, 64, TPG], f32)
            for q in range(4):
                # Stream-transpose quarter q of each tile (fp32), then split
                # into fp16 xh / xl' halves (contiguous writes only).
                xb = xbq.tile([P, TPG, QUARTER], f32)
                xhl = packp.tile([P, 2, TPG, QUARTER], f16, tag="xhl")
                for t in range(TPG):
                    nc.vector.transpose(
                        xb[:, t, :], xts[t][:, q * QUARTER : (q + 1) * QUARTER]
                    )
                    nc.scalar.copy(xhl[:, 0, t, :], xb[:, t, :])  # xh = fp16(xb)
                    # xl' = fp16((xb - xh) * 2048), via an fp32 residual
                    # scratch (prescale keeps xl' out of fp16-subnormal range).
                    rs = junkp.tile([P, QUARTER], f32, tag="resid")
                    nc.vector.tensor_sub(rs[:], xb[:, t, :], xhl[:, 0, t, :])
                    nc.scalar.mul(xhl[:, 1, t, :], rs[:], XL_SCALE)
                for cl in range(C2PQ):
                    c2 = q * C2PQ + cl
                    rhs = xhl[:, :, :, 64 * cl : 64 * (cl + 1)].transpose([0, 1, 3, 2])
                    nc.tensor.matmul(
                        u_ps[0:MW, :, :, :],
                        lhsT=wb_sb[:, c2, :],
                        rhs=rhs,
                        start=(c2 == 0),
                        stop=(c2 == N_C2 - 1),
                        skip_group_check=True,
                    )

            # Transpose each 64-wide quarter of u_ps to [(2j+t), (hl,e,a,l)].
            u_sb = small.tile([P, 2 * 64 * TPG], f32, tag="u_sb")
            nc.vector.tensor_copy(u_sb[0:MW, :], u_ps[0:MW, :, :, :])
            uts = []
            for q in range(4):
                utq = utps.tile([P, MW], f32, tag="utq")
                nc.tensor.transpose(
                    utq[0:64, :], u_sb[0:MW, 64 * q : 64 * (q + 1)], id_sb[0:MW, 0:MW]
                )
                uts.append(utq)

            # Valid slices: q0 (xh,j64<32,e=0): Wh [0:24], Wl' [48:72]
            #               q1 (xh,j64>=32,e=1): Wh [24:48], Wl' [72:96]
            #               q2 (xl',j<32,e=0): Wh [0:24]   (x2048)
            #               q3 (xl',j>=32,e=1): Wh [24:48] (x2048)
            s1 = small.tile([P, ML], f32, tag="s1")
            nc.vector.tensor_copy(s1[0:64, :], uts[0][0:64, 0:ML])
            nc.vector.tensor_add(s1[0:64, :], s1[0:64, :], uts[1][0:64, ML : 2 * ML])
            s2 = small.tile([P, ML], f32, tag="s2")
            nc.vector.tensor_copy(s2[0:64, :], uts[0][0:64, 2 * ML : 3 * ML])
            nc.vector.tensor_add(s2[0:64, :], s2[0:64, :], uts[1][0:64, 3 * ML : 4 * ML])
            nc.vector.tensor_add(s2[0:64, :], s2[0:64, :], uts[2][0:64, 0:ML])
            nc.vector.tensor_add(s2[0:64, :], s2[0:64, :], uts[3][0:64, ML : 2 * ML])
            # u1 = 1 + s1 + s2/2048  (per-(a,l) column, partitions (2j+t))
            u1 = small.tile([P, ML], f32, tag="u1")
            nc.vector.tensor_scalar(
                u1[0:64, :],
                s2[0:64, :],
                1.0 / XL_SCALE,
                1.0,
                mybir.AluOpType.mult,
                mybir.AluOpType.add,
            )
            nc.vector.tensor_add(u1[0:64, :], u1[0:64, :], s1[0:64, :])

            # c = prod_l (1 + u_l) (+ v terms), in [(2j+t), (a,l)].
            u1v = u1[:].rearrange("p (a l) -> p a l", a=4, l=L)
            ctr = small.tile([P, 4], f32, tag="ctr")
            if use_v:
                vbv = vb_sb[:].rearrange("p (a l) -> p a l", a=4, l=L)
                tmp = small.tile([P, 4], f32, tag="ctmp")
                nc.vector.tensor_add(ctr[0:64, :], u1v[0:64, :, 0], vbv[0:64, :, 0])
                for i in range(1, L):
                    nc.vector.tensor_mul(tmp[0:64, :], ctr[0:64, :], u1v[0:64, :, i])
                    nc.vector.tensor_add(ctr[0:64, :], tmp[0:64, :], vbv[0:64, :, i])
            else:
                m3 = small.tile([P, 4, 3], f32, tag="m3")
                nc.vector.tensor_mul(m3[0:64, :, 0], u1v[0:64, :, 0], u1v[0:64, :, 1])
                nc.vector.tensor_mul(m3[0:64, :, 1], u1v[0:64, :, 2], u1v[0:64, :, 3])
                nc.vector.tensor_mul(m3[0:64, :, 2], u1v[0:64, :, 4], u1v[0:64, :, 5])
                nc.vector.tensor_mul(ctr[0:64, :], m3[0:64, :, 0], m3[0:64, :, 1])
                nc.vector.tensor_mul(ctr[0:64, :], ctr[0:64, :], m3[0:64, :, 2])

            for t in range(TPG):
                # j-broadcast: jb[(a',j), a] = ctr[2j+t, a] for all a'.
                jb_ps = cps.tile([P, 4], f32)
                nc.tensor.matmul(
                    jb_ps[:],
                    lhsT=sel_sb[0:64, t, :],
                    rhs=ctr[0:64, :],
                    start=True,
                    stop=True,
                )
                # Diagonal pick: c_col[32a+j] = jb[(a,j), a] — four
                # partition-aligned copies (no cross-partition movement).
                c_col = small.tile([P, 1], f32, tag="c_col")
                for a in range(4):
                    nc.vector.tensor_copy(
                        c_col[32 * a : 32 * (a + 1), :],
                        jb_ps[32 * a : 32 * (a + 1), a : a + 1],
                    )
                # out = x0 * c (+ beta), in place, then store.
                xt = xts[t]
                if t % 2 == 0:
                    nc.scalar.mul(xt[:], xt[:], c_col[:, 0:1])
                else:
                    nc.vector.tensor_scalar_mul(xt[:], xt[:], c_col[:, 0:1])
                if use_b:
                    nc.vector.tensor_add(xt[:], xt[:], bb_sb[:])
                nc.sync.dma_start(
                    out[(g * TPG + t) * P : (g * TPG + t + 1) * P, :], xt[:]
                )

    nc.compile()
    return nc


def _get_program(use_v: bool, use_b: bool):
    key = (use_v, use_b)
    if key not in _prog_cache:
        _prog_cache[key] = _build_program(use_v, use_b)
    return _prog_cache[key]


# test.py reads this after a traced run to get exec_time_ns etc.
_last_results = None


def _host_prep(w_np: np.ndarray, b_np: np.ndarray):
    """Derive the device-side parameter tensors."""
    W = w_np[:, :, 0].T.astype(np.float32)  # [D, L]

    Wh = W.astype(np.float16)
    Wl = ((W.astype(np.float64) - Wh.astype(np.float64)) * XL_SCALE).astype(
        np.float16
    )

    # wb[(a,i), C2, (hl,e,a',l)] = (a==a') * W_hl[32*(2*C2+e)+i, l]
    wb = np.zeros((P, N_C2, MW), dtype=np.float16)
    for hl, Wx in enumerate((Wh, Wl)):
        Wc = Wx.reshape(N_C2, 2, 32, L)  # [C2, e, i, l]
        for e in range(2):
            base = hl * 2 * ML + e * ML
            for a in range(4):
                wb[32 * a : 32 * (a + 1), :, base + a * L : base + (a + 1) * L] = (
                    Wc[:, e].transpose(1, 0, 2)
                )

    # sel[(j',t'), t, (a',j)] = (t'==t) * (j==j'), partitions p = TPG*j' + t'
    p_idx = np.arange(P)
    jp, tp = p_idx // TPG, p_idx % TPG
    m_idx = np.arange(P)
    jm = m_idx % 32
    sel = np.zeros((P, TPG, P), dtype=np.float32)
    for t in range(TPG):
        sel[:, t, :] = ((tp[:, None] == t) & (jp[:, None] == jm[None, :])).astype(
            np.float32
        )

    ident = np.eye(P, dtype=np.float32)

    beta = np.zeros(D, dtype=np.float32)
    v = np.zeros(L, dtype=np.float32)
    for i in range(L):
        v[i] = float(beta @ W[:, i])
        beta = beta + b_np[i]
    return wb, sel, ident, v, beta


def kernel(inputs: np.ndarray, w: np.ndarray, b: np.ndarray) -> np.ndarray:
    import os

    from concourse.bass_utils import run_bass_kernel_spmd

    global _last_results

    x0 = np.ascontiguousarray(np.asarray(inputs, dtype=np.float32))
    w_np = np.asarray(w, dtype=np.float32)
    b_np = np.asarray(b, dtype=np.float32)
    assert x0.shape == (B, D) and w_np.shape == (L, D, 1) and b_np.shape == (L, D)

    wb, sel, ident, v, beta = _host_prep(w_np, b_np)

    use_v = bool(np.any(v != 0.0))
    use_b = bool(np.any(beta != 0.0))

    nc = _get_program(use_v, use_b)

    base = {"wb": wb, "sel": sel, "ident": ident}
    if use_v:
        # v broadcast to [(2j+t), (a,l)]: column (a,l) holds v[l].
        vbt = np.tile(v, 4)[None, :] * np.ones((P, 1), np.float32)
        base["vb"] = np.ascontiguousarray(vbt.astype(np.float32))
    if use_b:
        bb = np.broadcast_to(beta, (P, D)).astype(np.float32)
        base["bb"] = np.ascontiguousarray(bb)

    in_maps = [
        {**base, "x": x0[i * B_SHARD : (i + 1) * B_SHARD]} for i in range(N_CORES)
    ]

    trace = bool(int(os.environ.get("KERNEL_TRACE", "0")))
    res = run_bass_kernel_spmd(
        nc, in_maps, core_ids=list(range(N_CORES)), trace=trace
    )
    _last_results = res

    out = np.empty((B, D), dtype=np.float32)
    for i in range(N_CORES):
        out[i * B_SHARD : (i + 1) * B_SHARD] = res.results[i]["out"]
    return out

